# revision 1
# baseline (speedup 1.0000x reference)
"""Trainium2 Bass kernel for nn_Block_56667798504032 — head-parallel redesign.

Sharding: 8 cores = 4 batches x 2 head-halves. Each core computes LN1 + q/k/v
for its 8 heads over ALL 2048 tokens, causal attention for its 8 heads (uniform
static structure: q-block s needs k-tiles 0..4s+3, diagonal tiles narrowed),
then pairs exchange y via AllGather so each core runs LN2+MLP on its token half.

Key wins vs the v1 kernel:
- No duplicated k/v compute (was 2x per batch pair).
- Attention processes only causal k-tiles (40 tiles/head vs 56 equivalent),
  with diagonal tiles narrowed to their unmasked query range.
- Softmax denominators summed on DVE (PE does one 512-col matmul per (h,s)),
  normalization deferred past the exchange and folded into the LN2 scaling.
- k/q stay SBUF-resident (no DRAM bounce); per-head reciprocal serialization
  removed entirely.
"""
import math
import numpy as np

import concourse.bass as bass
import concourse.mybir as mybir
import concourse.tile as tile
from concourse import bacc
from concourse.bass_utils import run_bass_kernel_spmd

F32 = mybir.dt.float32
FP16 = mybir.dt.float16
AF = mybir.ActivationFunctionType
ALU = mybir.AluOpType

FULL_DIMS = dict(B=4, T=2048, C=2048, NH=16, HD=128)
EPS = 1e-5
N_CORES = 8
GROUPS = [[0, 1], [2, 3], [4, 5], [6, 7]]
# slot s holds global head 8*r + 2*g + hh  (g = s//4, r = (s%4)//2, hh = s%2)
SLOT_HEADS = [8 * r + 2 * g + hh for g in range(4) for r in range(2) for hh in range(2)]
DSCALE = 1.0 / 16.0  # fp16 headroom for stored softmax denominators


def build_nc(dims):
    DT = FP16
    B, T, C, NH, HD = dims["B"], dims["T"], dims["C"], dims["NH"], dims["HD"]
    assert HD == 128
    NHL = NH // 2           # heads per core
    TQ = T // 2             # tokens per core for MLP
    CH = C // 128
    CHA = CH + 1
    M4 = 4 * C
    MCH = M4 // 128
    MCHA = MCH + 1
    QS = 512
    NS = T // QS            # 4 query blocks
    NMT = M4 // 128
    NNT = C // 128
    PSUP = 16

    nc = bacc.Bacc(None, target_bir_lowering=False)
    with tile.TileContext(nc) as tc:
        with tc.tile_pool(name="dram", bufs=1, space="DRAM") as dram:
            xT = dram.tile([C, T], DT, kind="ExternalInput", uniquify=False, name="xT")
            wq = dram.tile([CHA * 128, NHL * 128], DT, kind="ExternalInput", uniquify=False, name="wq")
            wk = dram.tile([CHA * 128, NHL * 128], DT, kind="ExternalInput", uniquify=False, name="wk")
            wv = dram.tile([CHA * 128, NHL * 128], DT, kind="ExternalInput", uniquify=False, name="wv")
            wfc = dram.tile([CHA * 128, M4], DT, kind="ExternalInput", uniquify=False, name="wfc")
            wpr = dram.tile([MCHA * 128, C], DT, kind="ExternalInput", uniquify=False, name="wpr")
            msel = dram.tile([1, 2], F32, kind="ExternalInput", uniquify=False, name="msel")
            fccol = dram.tile([128, M4 // 128], F32, kind="ExternalInput", uniquify=False, name="fccol")
            fcbias = dram.tile([128, M4 // 128], F32, kind="ExternalInput", uniquify=False, name="fcbias")
            prbias = dram.tile([128, C // 128], F32, kind="ExternalInput", uniquify=False, name="prbias")
            outT = dram.tile([C, TQ], F32, kind="ExternalOutput", uniquify=False, name="outT")
            vdr = dram.tile([T // 128, 128, NHL * 128], DT, name="vdr")
            rcb = dram.tile([1, T], F32, name="rcb")
            yg_in = [dram.tile([2, 129, T], DT, name=f"ygin{g}") for g in range(4)]
            yg_out = [dram.tile([2, 2, 129, T], DT, name=f"ygout{g}") for g in range(4)]

            xT_r = xT[:].rearrange("(ch p) t -> p ch t", p=128)
            wq_r = wq[:].rearrange("(ch p) n -> p ch n", p=128)
            wk_r = wk[:].rearrange("(ch p) n -> p ch n", p=128)
            wv_r = wv[:].rearrange("(ch p) n -> p ch n", p=128)
            wfc_r = wfc[:].rearrange("(ch p) n -> p ch n", p=128)
            wpr_r = wpr[:].rearrange("(ch p) n -> p ch n", p=128)
            outT_r = outT[:].rearrange("(nt p) t -> p nt t", p=128)

            with tc.tile_pool(name="sb_top", bufs=1) as sbtop:
                ones1f = sbtop.tile([128, 1], F32, name="ones1f")
                nc.vector.memset(ones1f[:], 1.0)
                ones1 = sbtop.tile([128, 1], DT, name="ones1")
                nc.vector.tensor_copy(ones1[:], ones1f[:])
                eps_t = sbtop.tile([1, 1], F32, name="eps_t")
                nc.vector.memset(eps_t[:], EPS)
                # static causal mask for diagonal 128x128 tiles (col >= partition)
                dmask = sbtop.tile([128, 128], DT, name="dmask")
                nc.vector.memset(dmask[:], 1.0)
                nc.gpsimd.affine_select(dmask[:], dmask[:], [[1, 128]],
                                        ALU.is_ge, 0.0, base=0, channel_multiplier=-1)

                # ======== k/q stay resident through attention ========
                with tc.tile_pool(name="sb_kq", bufs=1) as sbkq:
                    ksb = sbkq.tile([128, NHL, T], DT, name="ksb")
                    qsb = sbkq.tile([128, NHL, T], DT, name="qsb")

                    with tc.tile_pool(name="sb_xs", bufs=1) as sbxs:
                        xs = sbxs.tile([128, CHA, T], DT, name="xs")
                        bc = sbxs.tile([128, T], DT, name="bc")
                        rstd_col = sbxs.tile([128, T // 128], F32, name="rstd_col")

                        # ---- Phase 0: x -> xs, LN1 stats, in-place prescale ----
                        with (
                            tc.tile_pool(name="p0_sb", bufs=1) as sb0,
                            tc.tile_pool(name="p0_st", bufs=8) as st0,
                            tc.tile_pool(name="p0_ps", bufs=2, space="PSUM") as ps0,
                        ):
                            for ch in range(CH):
                                nc.sync.dma_start(xs[:, ch, :], xT_r[:, ch, :])
                            sum_row = sb0.tile([1, T], F32, name="sum_row")
                            sq_row = sb0.tile([1, T], F32, name="sq_row")
                            rstd_row = sb0.tile([1, T], F32, name="rstd_row")
                            tmp_row = sb0.tile([1, T], F32, name="tmp_row")
                            # qkv runs on RAW x; rstd applied at PSUM eviction.
                            # aug row 0 = -mu (weight row: colsum); row 1 stays 1
                            # (its weight row is the folded bias, all-zero here,
                            # so eviction scaling cannot disturb it).
                            # The whole LN1 scalar chain is pipelined per
                            # 512-token slice so early bc/rstd_col slices
                            # unblock evictions before the last stats finish.
                            rstd_dt = sb0.tile([1, T], DT, name="rstd_dt")
                            nc.vector.memset(xs[:, CH, :], 0.0)
                            nc.vector.memset(xs[0:2, CH, :], 1.0)
                            for sl in range(T // 512):
                                ps_s = ps0.tile([1, 512], F32, name="ps_s", tag="ps_s")
                                ps_q = ps0.tile([1, 512], F32, name="ps_q", tag="ps_q")
                                csl = slice(sl * 512, (sl + 1) * 512)
                                for ch in range(CH):
                                    nc.tensor.matmul(ps_s[:], ones1[:], xs[:, ch, csl],
                                                     start=(ch == 0), stop=(ch == CH - 1))
                                    x2 = st0.tile([128, 512], DT, name="x2", tag="x2")
                                    nc.scalar.activation(x2[:], xs[:, ch, csl], AF.Square)
                                    nc.tensor.matmul(ps_q[:], ones1[:], x2[:],
                                                     start=(ch == 0), stop=(ch == CH - 1))
                                nc.scalar.copy(sum_row[:, csl], ps_s[:])
                                nc.scalar.copy(sq_row[:, csl], ps_q[:])
                                nc.scalar.mul(sum_row[:, csl], sum_row[:, csl], 1.0 / C)
                                nc.scalar.mul(sq_row[:, csl], sq_row[:, csl], 1.0 / C)
                                nc.vector.tensor_mul(tmp_row[:, csl], sum_row[:, csl],
                                                     sum_row[:, csl])
                                nc.vector.tensor_sub(sq_row[:, csl], sq_row[:, csl],
                                                     tmp_row[:, csl])
                                nc.scalar.activation(rstd_row[:, csl], sq_row[:, csl],
                                                     AF.Sqrt, bias=eps_t[:])
                                nc.vector.reciprocal(rstd_row[:, csl], rstd_row[:, csl])
                                nc.scalar.mul(tmp_row[:, csl], sum_row[:, csl], -1.0)
                                nc.vector.tensor_copy(rstd_dt[:, csl], rstd_row[:, csl])
                                nc.gpsimd.partition_broadcast(bc[:, csl], rstd_dt[:, csl])
                                nc.vector.tensor_copy(xs[0:1, CH, csl], tmp_row[:, csl])
                                nc.sync.dma_start(rcb[0:1, csl], rstd_row[:, csl])
                                for tt in range(4 * sl, 4 * sl + 4):
                                    nc.sync.dma_start(rstd_col[:, tt:tt + 1],
                                                      rcb[0:1, tt * 128:(tt + 1) * 128])

                        # ---- Phase 1: q/k/v projections for my 8 heads ----
                        with (
                            tc.tile_pool(name="p1_w", bufs=3) as wp1,
                            tc.tile_pool(name="p1_wv", bufs=2) as wvp1,
                            tc.tile_pool(name="p1_ev", bufs=4) as evp1,
                            tc.tile_pool(name="p1_ps", bufs=1, space="PSUM") as psk,
                            tc.tile_pool(name="p1_psv", bufs=3, space="PSUM") as psv1,
                        ):
                            for (wr, dst) in ((wk_r, ksb), (wq_r, qsb)):
                                for ot in range(NHL):
                                    wt = wp1.tile([128, CHA, 128], DT, name="wt", tag="wblk")
                                    nc.sync.dma_start(wt[:], wr[:, :, ot * 128:(ot + 1) * 128])
                                    pss_l = [psk.tile([128, 512], F32, name=f"pk{i}", tag=f"pk{i}")
                                             for i in range(4)]
                                    for ch in range(CHA):
                                        for sl in range(4):
                                            nc.tensor.matmul(
                                                pss_l[sl][:], wt[:, ch, :],
                                                xs[:, ch, sl * 512:(sl + 1) * 512],
                                                start=(ch == 0), stop=(ch == CHA - 1))
                                    for sl in range(4):
                                        nc.vector.tensor_mul(
                                            dst[:, ot, sl * 512:(sl + 1) * 512],
                                            pss_l[sl][:], bc[:, sl * 512:(sl + 1) * 512])
                            for osl in range(2):
                                wvsb = wvp1.tile([128, CHA, 512], DT, name="wvsb", tag="wvh")
                                nc.sync.dma_start(wvsb[:], wv_r[:, :, osl * 512:(osl + 1) * 512])
                                for tt in range(T // 128):
                                    psv = psv1.tile([128, 512], F32, name="psv", tag="psv")
                                    for ch in range(CHA):
                                        nc.tensor.matmul(
                                            psv[:],
                                            xs[:, ch, tt * 128:(tt + 1) * 128],
                                            wvsb[:, ch, :],
                                            start=(ch == 0), stop=(ch == CHA - 1))
                                    ev = evp1.tile([128, 512], DT, name="evv", tag="ev")
                                    nc.scalar.mul(ev[:], psv[:], rstd_col[:, tt:tt + 1])
                                    nc.sync.dma_start(
                                        vdr[tt, :, osl * 512:(osl + 1) * 512], ev[:])

                    # ---- Phase 2: causal attention per head + y export ----
                    with (
                        tc.tile_pool(name="p2_vh", bufs=2) as vhp,
                        tc.tile_pool(name="p2_es", bufs=6) as esp,
                        tc.tile_pool(name="p2_su", bufs=2) as sup,
                        tc.tile_pool(name="p2_ev", bufs=4) as evp2,
                        tc.tile_pool(name="p2_pss", bufs=4, space="PSUM") as pssp,
                        tc.tile_pool(name="p2_psy", bufs=2, space="PSUM") as psyp,
                        tc.tile_pool(name="p2_psd", bufs=2, space="PSUM") as psdp,
                    ):
                        for lh in range(NHL):
                            g, hh = lh // 2, lh % 2
                            vh = vhp.tile([128, T // 128, 128], DT, name="vh", tag="vh")
                            nc.sync.dma_start(
                                vh[:],
                                vdr[:].rearrange("t p n -> p t n")[:, :, lh * 128:(lh + 1) * 128])
                            for s in range(NS):
                                njt = 4 * s + 4
                                q0 = s * QS
                                psy = psyp.tile([128, QS], F32, name="psy", tag="psy")
                                esum = sup.tile([128, QS], F32, name="esum", tag="esum")
                                # software-pipelined: scores run 2 tiles ahead of
                                # the attn@v matmuls so exp (ACT) hides under PE
                                es_q = []

                                def emit_y(jj, es):
                                    dd = max(0, jj - 4 * s)
                                    w0 = 128 * dd
                                    nc.tensor.matmul(psy[:, w0:QS], vh[:, jj, :],
                                                     es[:, 0:QS - w0],
                                                     start=(jj == 0), stop=(jj == njt - 1))

                                for j in range(njt):
                                    dd = max(0, j - 4 * s)
                                    w0 = 128 * dd
                                    wid = QS - w0
                                    pss = pssp.tile([128, QS], F32, name="pss", tag="pss")
                                    nc.tensor.matmul(pss[:, 0:wid],
                                                     ksb[:, lh, j * 128:(j + 1) * 128],
                                                     qsb[:, lh, q0 + w0:q0 + QS],
                                                     start=True, stop=True)
                                    es = esp.tile([128, QS], DT, name="es", tag="es")
                                    nc.scalar.activation(es[:, 0:wid], pss[:, 0:wid], AF.Exp)
                                    if j >= 4 * s:
                                        nc.vector.tensor_mul(es[:, 0:128], es[:, 0:128],
                                                             dmask[:])
                                    if j == 0:
                                        nc.vector.tensor_copy(esum[:], es[:])
                                    else:
                                        nc.vector.tensor_add(esum[:, w0:QS], esum[:, w0:QS],
                                                             es[:, 0:wid])
                                    es_q.append((j, es))
                                    if len(es_q) > 2:
                                        emit_y(*es_q.pop(0))
                                for jj, es in es_q:
                                    emit_y(jj, es)
                                esum16 = sup.tile([128, QS], DT, name="esum16", tag="esum16")
                                nc.vector.tensor_copy(esum16[:], esum[:])
                                psd = psdp.tile([1, QS], F32, name="psd", tag="psd")
                                nc.tensor.matmul(psd[:], ones1[:], esum16[:],
                                                 start=True, stop=True)
                                drow = evp2.tile([1, QS], DT, name="drow", tag="drow")
                                nc.scalar.mul(drow[:], psd[:], DSCALE)
                                nc.sync.dma_start(yg_in[g][hh, 128:129, q0:q0 + QS], drow[:])
                                yev = evp2.tile([128, QS], DT, name="yev", tag="yev")
                                nc.vector.tensor_copy(yev[:], psy[:])
                                nc.sync.dma_start(yg_in[g][hh, 0:128, q0:q0 + QS], yev[:])
                            if hh == 1:
                                nc.gpsimd.collective_compute(
                                    "AllGather", ALU.bypass, replica_groups=GROUPS,
                                    ins=[yg_in[g][:].opt()], outs=[yg_out[g][:].opt()])

                # ---- Phase 3: select token half, LN2, MLP ----
                with tc.tile_pool(name="sb_y", bufs=1) as sby:
                    ysr = sby.tile([128, CH, TQ], DT, name="ysr")
                    fccol_s = sby.tile([128, NMT], F32, name="fccol_s")
                    fcbias_s = sby.tile([128, NMT], F32, name="fcbias_s")
                    prbias_s = sby.tile([128, NNT], F32, name="prbias_s")
                    nc.sync.dma_start(fccol_s[:], fccol[:])
                    nc.sync.dma_start(fcbias_s[:], fcbias[:])
                    nc.sync.dma_start(prbias_s[:], prbias[:])
                    bcnm = sby.tile([128, TQ], F32, name="bcnm")

                    with (
                        tc.tile_pool(name="p3_sb", bufs=1) as sb3,
                        tc.tile_pool(name="p3_st", bufs=3) as st3,
                        tc.tile_pool(name="p3_ro", bufs=4) as ro3,
                        tc.tile_pool(name="p3_ps", bufs=2, space="PSUM") as ps3,
                    ):
                        ysb = sb3.tile([128, NH, TQ], DT, name="ysb")
                        dstag = sb3.tile([16, T], DT, name="dstag")
                        inv_d = sb3.tile([16, TQ], F32, name="inv_d")
                        pmp = sb3.tile([128, TQ], F32, name="pmp")
                        pqp = sb3.tile([128, TQ], F32, name="pqp")
                        nc.vector.memset(pmp[:], 0.0)
                        nc.vector.memset(pqp[:], 0.0)
                        ms = sb3.tile([1, 2], F32, name="ms")
                        nc.sync.dma_start(ms[:], msel[:])
                        m0b = sb3.tile([128, 1], F32, name="m0b")
                        m1b = sb3.tile([128, 1], F32, name="m1b")
                        nc.gpsimd.partition_broadcast(m0b[:], ms[:, 0:1])
                        nc.gpsimd.partition_broadcast(m1b[:], ms[:, 1:2])

                        for slot in range(NH):
                            g, r, hh = slot // 4, (slot % 4) // 2, slot % 2
                            ystag = st3.tile([128, T], DT, name="ystag", tag="ystag")
                            nc.sync.dma_start(ystag[:], yg_out[g][r, hh, 0:128, :])
                            t0 = st3.tile([128, TQ], DT, name="t0", tag="t0")
                            t1 = st3.tile([128, TQ], DT, name="t1", tag="t1")
                            nc.vector.tensor_scalar(t0[:], ystag[:, 0:TQ], m0b[:], None, ALU.mult)
                            nc.vector.tensor_scalar(t1[:], ystag[:, TQ:T], m1b[:], None, ALU.mult)
                            nc.vector.tensor_add(ysb[:, slot, :], t0[:], t1[:])
                            nc.sync.dma_start(dstag[slot:slot + 1, :],
                                              yg_out[g][r, hh, 128:129, :])
                        # denominators: select half, reciprocal (incl 1/DSCALE fold)
                        d0 = sb3.tile([16, TQ], F32, name="d0")
                        d1 = sb3.tile([16, TQ], F32, name="d1")
                        nc.vector.tensor_scalar(d0[:], dstag[:, 0:TQ], m0b[0:16, :], None, ALU.mult)
                        nc.vector.tensor_scalar(d1[:], dstag[:, TQ:T], m1b[0:16, :], None, ALU.mult)
                        nc.vector.tensor_add(d0[:], d0[:], d1[:])
                        nc.vector.reciprocal(inv_d[:], d0[:])
                        nc.vector.tensor_scalar(inv_d[:], inv_d[:], DSCALE, None, ALU.mult)

                        # per-slot raw stats -> pmp/pqp rows via small DMAs
                        for slot in range(NH):
                            for half in range(2):
                                hsl = slice(half * 512, (half + 1) * 512)
                                ps_m = ps3.tile([1, 512], F32, name="ps_m", tag="ps_m")
                                ps_q2 = ps3.tile([1, 512], F32, name="ps_q2", tag="ps_q2")
                                y2 = st3.tile([128, 512], DT, name="y2", tag="y2")
                                nc.scalar.activation(y2[:], ysb[:, slot, hsl], AF.Square,
                                                     scale=1.0 / 16.0)
                                nc.tensor.matmul(ps_m[:], ones1[:], ysb[:, slot, hsl],
                                                 start=True, stop=True)
                                nc.tensor.matmul(ps_q2[:], ones1[:], y2[:],
                                                 start=True, stop=True)
                                rm = ro3.tile([1, 512], F32, name="rm", tag="rm")
                                rq = ro3.tile([1, 512], F32, name="rq", tag="rq")
                                nc.scalar.copy(rm[:], ps_m[:])
                                nc.scalar.copy(rq[:], ps_q2[:])
                                nc.sync.dma_start(pmp[slot:slot + 1, hsl], rm[:])
                                nc.sync.dma_start(pqp[slot:slot + 1, hsl], rq[:])

                        # normalized stats: nm = pm*inv_d ; nq = pq*inv_d^2
                        nc.vector.tensor_mul(pmp[0:16, :], pmp[0:16, :], inv_d[:])
                        nc.vector.tensor_mul(pqp[0:16, :], pqp[0:16, :], inv_d[:])
                        nc.vector.tensor_mul(pqp[0:16, :], pqp[0:16, :], inv_d[:])
                        nmp16 = sb3.tile([128, TQ], DT, name="nmp16")
                        nqp16 = sb3.tile([128, TQ], DT, name="nqp16")
                        nc.vector.tensor_copy(nmp16[:], pmp[:])
                        nc.vector.tensor_copy(nqp16[:], pqp[:])
                        m_row = sb3.tile([1, TQ], F32, name="m_row")
                        s_row = sb3.tile([1, TQ], F32, name="s_row")
                        for half in range(2):
                            hsl = slice(half * 512, (half + 1) * 512)
                            ps_m = ps3.tile([1, 512], F32, name="ps_m2", tag="ps_m")
                            ps_q2 = ps3.tile([1, 512], F32, name="ps_q22", tag="ps_q2")
                            nc.tensor.matmul(ps_m[:], ones1[:], nmp16[:, hsl], start=True, stop=True)
                            nc.tensor.matmul(ps_q2[:], ones1[:], nqp16[:, hsl], start=True, stop=True)
                            nc.scalar.copy(m_row[:, hsl], ps_m[:])
                            nc.scalar.copy(s_row[:, hsl], ps_q2[:])
                        t_row = sb3.tile([1, TQ], F32, name="t_row")
                        r2_row = sb3.tile([1, TQ], F32, name="r2_row")
                        nm2_row = sb3.tile([1, TQ], F32, name="nm2_row")
                        nc.scalar.mul(m_row[:], m_row[:], 1.0 / C)
                        nc.scalar.mul(s_row[:], s_row[:], 256.0 / C)
                        nc.vector.tensor_mul(t_row[:], m_row[:], m_row[:])
                        nc.vector.tensor_sub(s_row[:], s_row[:], t_row[:])
                        nc.scalar.activation(r2_row[:], s_row[:], AF.Sqrt, bias=eps_t[:])
                        nc.vector.reciprocal(r2_row[:], r2_row[:])
                        nc.vector.tensor_mul(nm2_row[:], m_row[:], r2_row[:])
                        nc.scalar.mul(nm2_row[:], nm2_row[:], -1.0)

                        # scale16[slot,t] = rstd2[t]*inv_d[slot,t]; apply per slot
                        sc_b = sb3.tile([16, TQ], F32, name="sc_b")
                        nc.gpsimd.partition_broadcast(sc_b[:], r2_row[:])
                        scale16 = sb3.tile([16, TQ], DT, name="scale16")
                        nc.vector.tensor_mul(sc_b[:], sc_b[:], inv_d[:])
                        nc.vector.tensor_copy(scale16[:], sc_b[:])
                        swide = sb3.tile([1, NH * TQ], DT, name="swide")
                        nc.sync.dma_start(swide[:], scale16[:])
                        for slot in range(NH):
                            bsc = st3.tile([128, TQ], DT, name="bsc", tag="bsc")
                            nc.gpsimd.partition_broadcast(
                                bsc[:], swide[:, slot * TQ:(slot + 1) * TQ])
                            nc.vector.tensor_mul(ysr[:, slot, :], ysb[:, slot, :], bsc[:])
                        # broadcast -mu*rstd2 across partitions for the fc rank-1 fix
                        nc.gpsimd.partition_broadcast(bcnm[:], nm2_row[:])

                    # ---- Phase 4: MLP (as v1) ----
                    for ts in range(TQ // QS):
                        tsl = slice(ts * QS, (ts + 1) * QS)
                        with tc.tile_pool(name=f"p6_act{ts}", bufs=1) as sb6:
                            act = sb6.tile([128, MCH, QS], DT, name="act")
                            with (
                                tc.tile_pool(name=f"p6f_w{ts}", bufs=4) as wf6,
                                tc.tile_pool(name=f"p6f_t{ts}", bufs=3) as tf6,
                                tc.tile_pool(name=f"p6f_ps{ts}", bufs=3, space="PSUM") as psf6,
                            ):
                                for mt in range(NMT):
                                    wt = wf6.tile([128, CH, 128], DT, name="wt6", tag="w6")
                                    nc.sync.dma_start(
                                        wt[:], wfc_r[:, 0:CH, mt * 128:(mt + 1) * 128])
                                    psf = psf6.tile([128, QS], F32, name="psf", tag="psf")
                                    for ch in range(CH):
                                        nc.tensor.matmul(
                                            psf[:], wt[:, ch, :], ysr[:, ch, tsl],
                                            start=(ch == 0), stop=(ch == CH - 1))
                                    # rank-1 LN2 fold: (bcnm*colsum) + psum, then gelu(+bias)
                                    tmp = tf6.tile([128, QS], DT, name="tmp6", tag="t6")
                                    nc.vector.scalar_tensor_tensor(
                                        tmp[:], bcnm[:, tsl], fccol_s[:, mt:mt + 1],
                                        psf[:], ALU.mult, ALU.add)
                                    nc.scalar.activation(act[:, mt, :], tmp[:], AF.Gelu,
                                                         bias=fcbias_s[:, mt:mt + 1])
                            with (
                                tc.tile_pool(name=f"p6p_w{ts}", bufs=4) as wp6,
                                tc.tile_pool(name=f"p6p_ev{ts}", bufs=3) as evp6,
                                tc.tile_pool(name=f"p6p_ps{ts}", bufs=3, space="PSUM") as psp6,
                            ):
                                nsup = MCH // PSUP
                                for nt in range(NNT):
                                    pso = psp6.tile([128, QS], F32, name="pso", tag="pso")
                                    for sp in range(nsup):
                                        c0 = sp * PSUP
                                        c1 = c0 + PSUP
                                        wp = wp6.tile([128, PSUP, 128], DT, name="wp6",
                                                      tag="wp6")
                                        nc.sync.dma_start(
                                            wp[:], wpr_r[:, c0:c1, nt * 128:(nt + 1) * 128])
                                        for ch in range(c0, c1):
                                            nc.tensor.matmul(
                                                pso[:], wp[:, ch - c0, :], act[:, ch, :],
                                                start=(ch == 0), stop=(ch == MCH - 1))
                                    ev = evp6.tile([128, QS], F32, name="evo", tag="evo")
                                    nc.vector.tensor_scalar(ev[:], pso[:],
                                                            prbias_s[:, nt:nt + 1], None,
                                                            ALU.add)
                                    nc.sync.dma_start(outT_r[:, nt, tsl], ev[:])

    nc.compile()
    return nc


# ============ host side ============
_NC_CACHE = {}


def _get_nc(dims):
    key = tuple(sorted(dims.items()))
    if key not in _NC_CACHE:
        _NC_CACHE[key] = build_nc(dims)
    return _NC_CACHE[key]


def prep_weights(dims, ln1_w, ln1_b, attn_w, attn_b, ln2_w, ln2_b, fc_w, fc_b,
                 proj_w, proj_b):
    C = dims["C"]
    M4 = 4 * C
    CHA = C // 128 + 1
    MCHA = M4 // 128 + 1
    smscale = np.float32(1.0 / math.sqrt(dims["HD"]))

    def aug(wpart, bpart, g, bvec, scale=1.0):
        ncols = wpart.shape[1]
        out = np.zeros((CHA * 128, ncols), np.float32)
        wt = (g[:, None] * wpart).astype(np.float32)
        out[:C] = wt
        out[C] = wt.sum(0)
        out[C + 1] = bvec @ wpart + bpart
        return np.ascontiguousarray(out * np.float32(scale))

    # per-parity q/k/v slices (parity p owns global heads p*8..p*8+7)
    wq_p, wk_p, wv_p = [], [], []
    half = C // 2
    for p in range(2):
        cs = slice(p * half, (p + 1) * half)
        wq_p.append(aug(attn_w[:, 0:C][:, cs], attn_b[0:C][cs], ln1_w, ln1_b, smscale))
        wk_p.append(aug(attn_w[:, C:2 * C][:, cs], attn_b[C:2 * C][cs], ln1_w, ln1_b))
        wv_p.append(aug(attn_w[:, 2 * C:3 * C][:, cs], attn_b[2 * C:3 * C][cs], ln1_w, ln1_b))

    wfc = aug(fc_w, fc_b, ln2_w, ln2_b)
    # permute contraction row-blocks to the y slot order
    perm = np.concatenate([np.arange(h * 128, (h + 1) * 128) for h in SLOT_HEADS])
    wfc[:C] = wfc[:C][perm]
    # aug rows pulled out as per-partition columns (device applies them
    # as a rank-1 DVE fix + activation bias instead of extra matmul chunks)
    fccol = np.ascontiguousarray(wfc[C].reshape(M4 // 128, 128).T.astype(np.float32))
    fcbias = np.ascontiguousarray(wfc[C + 1].reshape(M4 // 128, 128).T.astype(np.float32))
    prbias = np.ascontiguousarray(np.asarray(proj_b, np.float32).reshape(C // 128, 128).T)
    wpr = np.zeros((MCHA * 128, C), np.float32)
    wpr[:M4] = proj_w
    wpr[M4] = proj_b
    return wq_p, wk_p, wv_p, np.ascontiguousarray(wfc), np.ascontiguousarray(wpr), \
        fccol, fcbias, prbias


def kernel(x, ln1_w, ln1_b, attn_w, attn_b, ln2_w, ln2_b, fc_w, fc_b, proj_w,
           proj_b, dims=None, n_cores=None, trace=False):
    dims = dims or FULL_DIMS
    n_cores = n_cores if n_cores is not None else N_CORES
    B, T, C = dims["B"], dims["T"], dims["C"]
    TQ = T // 2
    x = np.asarray(x, np.float32)
    args = [np.asarray(a, np.float32) for a in
            (ln1_w, ln1_b, attn_w, attn_b, ln2_w, ln2_b, fc_w, fc_b, proj_w, proj_b)]
    wq_p, wk_p, wv_p, wfc, wpr, fccol, fcbias, prbias = prep_weights(dims, *args)
    cast = lambda a: np.ascontiguousarray(a.astype(np.float16))
    wq_p = [cast(a) for a in wq_p]
    wk_p = [cast(a) for a in wk_p]
    wv_p = [cast(a) for a in wv_p]
    wfc, wpr = cast(wfc), cast(wpr)
    nc = _get_nc(dims)

    in_maps = []
    for c in range(n_cores):
        b, p = c // 2, c % 2
        xt = np.ascontiguousarray(x[b].T)
        in_maps.append({
            "xT": cast(xt), "wq": wq_p[p], "wk": wk_p[p], "wv": wv_p[p],
            "wfc": wfc, "wpr": wpr,
            "fccol": fccol, "fcbias": fcbias, "prbias": prbias,
            "msel": np.array([[1.0, 0.0]] if p == 0 else [[0.0, 1.0]], np.float32),
        })

    res = run_bass_kernel_spmd(nc, in_maps, core_ids=list(range(n_cores)), trace=trace)
    out = np.empty((B, T, C), np.float32)
    for c in range(n_cores):
        b, p = c // 2, c % 2
        out[b, p * TQ:(p + 1) * TQ, :] = res.results[c]["outT"].T
    if trace:
        return out, res
    return out

